# revision 34
# baseline (speedup 1.0000x reference)
"""HQQ 1-bit quantized linear (out = x @ dequant(W).T + bias) on 8 Trainium2
NeuronCores.

Sharding: 8-way row-parallel. x rows (M=8192) split into 8 shards of 1024;
every core computes its [1024, 4096] output slab against the FULL weight
matrix (K=4096 contraction, O=4096 out features). x is read from HBM exactly
once across the 8 cores (16 MiB/core), keeping DMA far below the PE roofline.

Per core:
  - x shard resident in SBUF as bf16 [128, 32 kt, 1024], cast f32->bf16 on
    the Scalar engine (keeps DVE free for dequant), loaded in half-tiles so
    the first cast lands early,
  - weights streamed per o-chunk of 512 out-features and dequantized on DVE
    (shift/and bit-extract, B*s, +(-z*s)) into per-k-tile bf16 tiles
    [128, 512] — one tile per k-tile so the tensor engine's dependency is
    per k-tile, not per chunk (kills the startup stall),
  - chunk 0 runs t-outer (all 8 m-tiles per dequantized k-tile, one PSUM
    bank each) so the PE starts as soon as the first k-tile is ready;
    steady chunks run m-tile-outer (proven gap-free),
  - PSUM drains on DVE with fused bias add (bias host-broadcast, bf16).

Host-side work is layout/packing only: transpose/permute/replicate/slice,
int16 container cast for the packed bytes, bf16 cast + per-group (-z*s)
product for the tiny [4096, 64] coefficient arrays.
"""

import sys

for _p in ("/opt/trn_rl_repo", "/root/.axon_site/_ro/trn_rl_repo"):
    if _p not in sys.path:
        sys.path.append(_p)

import numpy as np

P = 128
OC = 512                      # out-feature chunk per dequant/matmul round
NBITS_PER_BYTE = 8
GROUP_SIZE = 64
M_FULL, K_IN, O_FULL = 8192, 4096, 4096
N_CORES = 8
M_SH = M_FULL // N_CORES      # 1024 rows per core

_compiled = {}


def _build_nc():
    import concourse.bacc as bacc
    import concourse.mybir as mybir
    import concourse.tile as tile

    f32 = mybir.dt.float32
    bf16 = mybir.dt.bfloat16
    i16 = mybir.dt.int16

    PB = K_IN // NBITS_PER_BYTE   # 512 packed-byte rows
    N_KT = K_IN // P              # 32 k-tiles
    N_V = PB // P                 # 4 byte-tiles
    N_MT = M_SH // P              # 8 m-tiles
    N_OC = O_FULL // OC           # 8 o-chunks
    XH = N_KT // 2                # x loaded in k-halves

    nc = bacc.Bacc("TRN2", target_bir_lowering=False, debug=False,
                   num_devices=N_CORES)

    xt_d = nc.dram_tensor("xt", [K_IN, M_SH], f32, kind="ExternalInput")
    wpt_d = nc.dram_tensor("wpt", [PB, O_FULL], i16, kind="ExternalInput")
    sexp_d = nc.dram_tensor("sexp", [PB, O_FULL], bf16, kind="ExternalInput")
    nzs_d = nc.dram_tensor("nzs", [PB, O_FULL], bf16, kind="ExternalInput")
    bias_d = nc.dram_tensor("bias", [P, O_FULL], bf16, kind="ExternalInput")
    out_d = nc.dram_tensor("out", [M_SH, O_FULL], f32, kind="ExternalOutput")

    with tile.TileContext(nc) as tc:
        with tc.tile_pool(name="fixed", bufs=1) as fixed, \
             tc.tile_pool(name="xtf", bufs=2) as xtf_pool, \
             tc.tile_pool(name="wload", bufs=2) as wload_pool, \
             tc.tile_pool(name="deq", bufs=3) as deq_pool, \
             tc.tile_pool(name="wt", bufs=2 * N_KT) as wt_pool, \
             tc.tile_pool(name="outp", bufs=2) as out_pool, \
             tc.tile_pool(name="psum", bufs=8, space="PSUM") as psum_pool:

            def load_chunk(oc, split=False):
                osl = slice(oc * OC, (oc + 1) * OC)
                wpt_t = wload_pool.tile([P, N_V, OC], i16, tag="wpt", name="wpt_t")
                s_t = wload_pool.tile([P, N_V, OC], bf16, tag="s", name="s_t")
                nzs_t = wload_pool.tile([P, N_V, OC], bf16, tag="nzs", name="nzs_t")
                if split:
                    # per-byte-tile DMAs so dequant of the v=0 k-tiles can
                    # start as soon as the first (small) triple lands
                    for v in range(N_V):
                        vsl = slice(v * P, (v + 1) * P)
                        nc.sync.dma_start(wpt_t[:, v, :], wpt_d[vsl, osl])
                        nc.sync.dma_start(s_t[:, v, :], sexp_d[vsl, osl])
                        nc.sync.dma_start(nzs_t[:, v, :], nzs_d[vsl, osl])
                else:
                    nc.sync.dma_start(
                        wpt_t[:], wpt_d[:, osl].rearrange("(v p) o -> p v o", p=P))
                    nc.sync.dma_start(
                        s_t[:], sexp_d[:, osl].rearrange("(v p) o -> p v o", p=P))
                    nc.sync.dma_start(
                        nzs_t[:], nzs_d[:, osl].rearrange("(v p) o -> p v o", p=P))
                return wpt_t, s_t, nzs_t, osl

            # chunk-0 weight loads issued first: dequant of k-tile 0 is the
            # critical path to the first matmul.
            chunk0 = load_chunk(0)

            # ---- resident x shard: bf16 [128, N_KT, M_SH], half-tile loads
            # cast on ScalarE so the first piece lands fast ----
            xb = fixed.tile([P, N_KT, M_SH], bf16, tag="xb")
            for h in range(2):
                for mi in range(N_MT):
                    nq = 1
                    for q in range(nq):
                        qh = XH // nq
                        k0 = h * XH + q * qh
                        ksl = slice(k0, k0 + qh)
                        xt_f = xtf_pool.tile([P, XH, P], f32, tag="xtf",
                                             name="xt_f")
                        nc.sync.dma_start(
                            xt_f[:, :qh, :],
                            xt_d[k0 * P:(k0 + qh) * P, mi * P:(mi + 1) * P]
                            .rearrange("(t p) m -> p t m", p=P))
                        nc.scalar.copy(xb[:, ksl, mi * P:(mi + 1) * P],
                                       xt_f[:, :qh, :])

            # bias pre-broadcast on host: [128, O_FULL] bf16 (needed only at
            # the first drain, so loaded after the critical-path DMAs)
            bias_bc = fixed.tile([P, O_FULL], bf16, tag="biasbc")
            nc.sync.dma_start(bias_bc[:], bias_d[:, :])

            # ---- software pipeline over o-chunks: dequant for chunk c+2 is
            # issued AFTER chunk c's drains, so on the DVE FIFO each chunk's
            # dequant runs during the PREVIOUS chunk's matmuls and the drains
            # still execute promptly at each chunk boundary. ----
            deqs = {}

            # chunk 0 dequants in v-major order (all byte-tile-0 k-tiles
            # first) so only the first small load triple gates the PE start;
            # steady chunks use natural order.
            T_NAT = list(range(N_KT))
            T_VMAJ = [u * N_V + v for v in range(N_V) for u in range(N_KT // N_V)]

            def issue_deq(oc, preloaded=None):
                wpt_t, s_t, nzs_t, osl = preloaded or load_chunk(oc)
                # dequant per k-tile into its own tile: matmul dependencies
                # are per k-tile, so the PE starts on WT[0] immediately.
                wt_tiles = {}
                for t in T_NAT:
                    u, v = t // N_V, t % N_V
                    B_t = deq_pool.tile([P, OC], i16, tag="B", name="B_t")
                    nc.vector.tensor_scalar(
                        B_t[:], wpt_t[:, v, :], u, 1,
                        mybir.AluOpType.logical_shift_right,
                        mybir.AluOpType.bitwise_and)
                    bs_t = deq_pool.tile([P, OC], bf16, tag="bs", name="bs_t")
                    nc.vector.scalar_tensor_tensor(
                        bs_t[:], B_t[:], 1.0, s_t[:, v, :],
                        mybir.AluOpType.mult, mybir.AluOpType.mult)
                    WT_t = wt_pool.tile([P, OC], bf16, tag="WT", name=f"WT{t}")
                    nc.vector.tensor_tensor(WT_t[:], bs_t[:], nzs_t[:, v, :],
                                            mybir.AluOpType.add)
                    wt_tiles[t] = WT_t
                deqs[oc] = (wt_tiles, osl)

            issue_deq(0, chunk0)
            issue_deq(1)
            for oc in range(N_OC):
                wt_tiles, osl = deqs.pop(oc)

                def drain(ps, mi):
                    out_t = out_pool.tile([P, OC], f32, tag="out", name="out_t")
                    nc.vector.tensor_tensor(out_t[:], ps[:], bias_bc[:, osl],
                                            mybir.AluOpType.add)
                    nc.sync.dma_start(out_d[mi * P:(mi + 1) * P, osl], out_t[:])

                if oc <= 1:
                    # t-outer: PE consumes each k-tile right as dequant lands
                    # (chunks 0 and 1 run while the DVE dequant pipeline is
                    # still warming; 8 matmuls per k-tile > dequant pace)
                    pss = [psum_pool.tile([P, OC], f32, tag="ps", name="ps")
                           for _ in range(N_MT)]
                    for i, t in enumerate(T_NAT):
                        for mi in range(N_MT):
                            nc.tensor.matmul(
                                pss[mi][:], xb[:, t, mi * P:(mi + 1) * P],
                                wt_tiles[t][:],
                                start=(i == 0), stop=(i == N_KT - 1))
                    for mi in range(N_MT):
                        drain(pss[mi], mi)
                else:
                    # mi-outer steady loop (gap-free)
                    for mi in range(N_MT):
                        ps = psum_pool.tile([P, OC], f32, tag="ps", name="ps")
                        for t in range(N_KT):
                            nc.tensor.matmul(
                                ps[:], xb[:, t, mi * P:(mi + 1) * P],
                                wt_tiles[t][:],
                                start=(t == 0), stop=(t == N_KT - 1))
                        drain(ps, mi)
                if oc + 2 < N_OC:
                    issue_deq(oc + 2)
    nc.compile()
    return nc


def _get_nc(**kw):
    key = tuple(sorted(kw.items()))
    if key not in _compiled:
        _compiled[key] = _build_nc(**kw)
    return _compiled[key]


def _host_prep(x, W_packed, scale, zero, bias):
    """Layout/packing-only prep of per-core input maps."""
    import ml_dtypes
    bf16 = ml_dtypes.bfloat16
    PB = K_IN // NBITS_PER_BYTE
    NG = K_IN // GROUP_SIZE
    x = np.asarray(x, dtype=np.float32)
    W_packed = np.asarray(W_packed)
    scale2d = np.asarray(scale, dtype=np.float32).reshape(O_FULL, NG)
    zero2d = np.asarray(zero, dtype=np.float32).reshape(O_FULL, NG)
    bias = np.asarray(bias, dtype=np.float32)

    # shared (replicated) tensors
    wpt = np.ascontiguousarray(W_packed.T.astype(np.int16))          # [PB, O]
    sexp = np.ascontiguousarray(
        np.repeat(scale2d.T, NBITS_PER_BYTE, axis=0).astype(bf16))   # [PB, O]
    nzs = np.ascontiguousarray(
        np.repeat((-zero2d * scale2d).T, NBITS_PER_BYTE, axis=0).astype(bf16))
    bias2 = np.ascontiguousarray(
        np.broadcast_to(bias.astype(bf16)[None, :], (P, O_FULL)))

    in_maps = []
    for c in range(N_CORES):
        xs = x[c * M_SH:(c + 1) * M_SH]                              # [M_SH, K]
        # bit-plane-major permuted transpose: xt[u*PB + pb, m] = x[m, 8*pb + u]
        xt = xs.T.reshape(PB, NBITS_PER_BYTE, M_SH)
        xt = np.ascontiguousarray(xt.transpose(1, 0, 2).reshape(K_IN, M_SH))
        in_maps.append(dict(xt=xt, wpt=wpt, sexp=sexp, nzs=nzs, bias=bias2))
    return in_maps


def run_sharded(x, W_packed, scale, zero, bias, trace=False, **run_kwargs):
    """Compile (cached), run on 8 cores, return (full_out, BassKernelResults)."""
    from concourse.bass_utils import run_bass_kernel_spmd

    nc = _get_nc()
    in_maps = _host_prep(x, W_packed, scale, zero, bias)
    res = run_bass_kernel_spmd(nc, in_maps, core_ids=list(range(N_CORES)),
                               trace=trace, **run_kwargs)
    out = np.empty((M_FULL, O_FULL), dtype=np.float32)
    for c in range(N_CORES):
        out[c * M_SH:(c + 1) * M_SH, :] = res.results[c]["out"]
    return out, res


def kernel(x, W_packed, scale, zero, bias):
    out, _ = run_sharded(x, W_packed, scale, zero, bias)
    return out
